# revision 1
# baseline (speedup 1.0000x reference)
"""Trainium2 Bass kernel for nn_DistributionalQNetwork (C51 distributional Q).

Self-contained: hardcodes shapes from the problem spec.
  MLP: [B,1092] -> 512 -> 256 -> 128 -> 101 logits -> softmax
  C51 categorical projection with scatter-add into [B,101].

Pure data parallel across 8 NeuronCores (B=65536 -> 8192 rows/core), one
identical Bass program per core, inputs sharded on host, no collectives.

Device pipeline (per core, feature-major activations [feat, batch]):
  - All MLP inputs/weights stream as fp8e4m3 (weights scaled by 64 for
    subnormal headroom, undone by each relu's scale=1/64). Layers 1-3 use
    DoubleRow perf mode (2 k-rows/cell/cycle). obs cols 0..1023 are
    pre-transposed on host to [1024, B] fp8; the 68-feature tail
    (obs 1024.. + actions) plus a constant-1 row that carries b1 ride in a
    [128, B] fp8 tensor. b2 is added via the relu bias AP, b3 via a k=1
    ones-matmul rider, b4 via a k=1 ones-matmul. L4 un-transposes by using
    x3 as lhsT. L3's relu runs on DVE to offload ACT.
  - Softmax: one fused exp [128, 4*102] -> fp16 (logits span ~±0.3, no max
    subtraction; the pad column holds logit -30 so it vanishes); row sums
    via one DVE tensor_reduce.
  - C51 projection: b = clip(r + g*z, ±10)/dz is monotone per row, so equal
    target bins form contiguous runs (clip plateaus included -- the b==0 /
    b==100 piles are just the first/last runs; no separate masks). Host
    ships, per row: lw = b - floor(b) (fp16), the run-continuation gate E
    (fp16 0/1), and run-end scatter indices idxl/idxu (int16, -1 elsewhere).
    Device: p = e*inv (fp16 2x), wu = p*lw, wl = p-wu as chunk-wide [128,
    4*102] ops, then ONE tensor_tensor_scan per row-tile per weight array
    (y[t] = E[t]*y[t-1] + w[t]) produces run sums at run-end positions;
    GPSIMD local_scatters place them; one chunk-wide fp16 add combines.
    No bit-exact host replication is needed anywhere: the projection is
    continuous in b, and the reference's exact-integer-b double-mass quirk
    (~1e-5 of elements) is accepted as error (~3e-3 rel-fro, tol 2e-2).
  - g==0 rows (bootstrap==0) scatter nothing (idx=-1); the host adds their
    closed-form 2-bin output (independent of the MLP) afterwards.

Schedule: 2-deep software pipeline -- iteration n emits the projection
stage of chunk n-2 before the MLP stage of chunk n, so every engine queue
always holds ready work and cross-engine handoff latency is hidden. PSUM
is split 4+2+1+1 banks with the L1/L2 pools double-buffered. All side
inputs are packed [128, ...] on host so every DMA moves >=1.6KB contiguous
per partition (full DMA bus efficiency), loaded per 2048-row super-chunk
on the SP queue; output stores go through the Pool SWDGE queue so they
never block loads. Output is fp16 [128, n_tiles*101], unpacked on host.
"""
import os
import numpy as np
import ml_dtypes

import concourse.bacc as bacc
import concourse.mybir as mybir
from concourse import tile
from concourse.bass_utils import run_bass_kernel_spmd

F32 = np.float32
FP8 = ml_dtypes.float8_e4m3
BF16 = ml_dtypes.bfloat16
FP16 = np.float16

f32 = mybir.dt.float32
bf16 = mybir.dt.bfloat16
fp16 = mybir.dt.float16
i16 = mybir.dt.int16
f8 = mybir.dt.float8e4

Alu = mybir.AluOpType
Act = mybir.ActivationFunctionType
AX = mybir.AxisListType
DR = mybir.MatmulPerfMode.DoubleRow

B_FULL = 65536
N_CORES = 8
B_CORE = B_FULL // N_CORES      # 8192
D_OBS = 1090
H1, H2, H3 = 512, 256, 128
NA = 101
NA2 = 102                       # padded atom count (scan/scatter width)
TILE = 128
CHUNK = 512                     # batch rows per matmul sweep
SUPER = 1024                    # batch rows per DMA super-load
WSCALE = F32(64.0)              # fp8 weight scale (subnormal headroom)
R1CUT = 640                     # relu1b cols on ACT (rest on DVE)


def build_nc(n_rows=B_CORE):
    """Build the single-core Bass program (replicated over all cores)."""
    assert n_rows % SUPER == 0
    n_chunks = n_rows // CHUNK
    hpc = SUPER // CHUNK            # chunks per super-load
    assert hpc == 2                 # the DR-tail trick reads whole supers
    tps = SUPER // TILE             # row-tiles per super-load
    nt = n_rows // TILE
    kblk = tps * NA2                # one block (l/u or lw/E) per super

    nc = bacc.Bacc("TRN2", target_bir_lowering=False, debug=False)

    # ---- DRAM I/O ----
    xt8_d = nc.dram_tensor("xt8", [1024, n_rows], f8, kind="ExternalInput")
    tl8_d = nc.dram_tensor("tail8", [TILE, n_rows], f8, kind="ExternalInput")
    w1f8_d = nc.dram_tensor("w1f8", [TILE, 4096], f8, kind="ExternalInput")
    w1t8_d = nc.dram_tensor("w1t8", [TILE, 3 * H1], f8, kind="ExternalInput")
    w2f8_d = nc.dram_tensor("w2f8", [TILE, 1024], f8, kind="ExternalInput")
    w3f8_d = nc.dram_tensor("w3f8", [TILE, 256], f8, kind="ExternalInput")
    w4p_d = nc.dram_tensor("w4p", [TILE, NA2], bf16, kind="ExternalInput")
    b4r_d = nc.dram_tensor("b4r", [1, NA2], bf16, kind="ExternalInput")
    b2c_d = nc.dram_tensor("b2c", [TILE, 2], f32, kind="ExternalInput")
    b3r_d = nc.dram_tensor("b3c64", [TILE, 1], f32, kind="ExternalInput")
    # [128, nsuper, {l,u} x tps x NA2] / [128, nsuper, {lw,E} x tps x NA2]
    idx_d = nc.dram_tensor("idxpk", [TILE, nt * 2 * NA2], i16,
                           kind="ExternalInput")
    lwe_d = nc.dram_tensor("lwepk", [TILE, nt * 2 * NA2], fp16,
                           kind="ExternalInput")
    out_d = nc.dram_tensor("out", [TILE, nt * NA], fp16, kind="ExternalOutput")

    with tile.TileContext(nc) as tc:
        with (
            tc.tile_pool(name="const", bufs=1) as cpool,
            tc.tile_pool(name="xin", bufs=4) as xpool,
            tc.tile_pool(name="acts", bufs=2) as apool,
            tc.tile_pool(name="x3p", bufs=2) as x3pool,
            tc.tile_pool(name="proj", bufs=2) as ppool,
            tc.tile_pool(name="outp", bufs=2) as opool,
            tc.tile_pool(name="ps1", bufs=2, space="PSUM") as ps1pool,
            tc.tile_pool(name="ps2", bufs=2, space="PSUM") as ps2pool,
            tc.tile_pool(name="ps3", bufs=1, space="PSUM") as ps3pool,
            tc.tile_pool(name="psl", bufs=1, space="PSUM") as pslpool,
        ):
            def xhalf_load(sb, h):
                xh = xpool.tile([TILE, 8 * CHUNK], f8, tag="xbh")
                r0 = sb * SUPER + h * CHUNK
                nc.sync.dma_start(
                    xh[:].rearrange("q (p i n) -> q p i n", p=4, i=2),
                    xt8_d[:, r0:r0 + CHUNK].rearrange(
                        "(p i q) n -> q p i n", p=4, i=2))
                return xh

            def loads(sb, xh0=None):
                if xh0 is None:
                    xh0 = xhalf_load(sb, 0)
                tl8 = xpool.tile([TILE, SUPER], f8, tag="tl8")
                nc.sync.dma_start(
                    tl8[:], tl8_d[:, sb * SUPER:(sb + 1) * SUPER])
                xh1 = xhalf_load(sb, 1)
                idxt = xpool.tile([TILE, 2 * kblk], i16, tag="idxt")
                nc.sync.dma_start(
                    idxt[:], idx_d[:, sb * 2 * kblk:(sb + 1) * 2 * kblk])
                lwet = xpool.tile([TILE, 2 * kblk], fp16, tag="lwet")
                nc.sync.dma_start(
                    lwet[:], lwe_d[:, sb * 2 * kblk:(sb + 1) * 2 * kblk])
                outsup = opool.tile([TILE, tps * NA], fp16, tag="outsup")
                return ((xh0, xh1), tl8, idxt, lwet, outsup)

            # ---- constants; w1 + first chunk's input first ----
            w1f8t = cpool.tile([TILE, 4096], f8)
            nc.sync.dma_start(w1f8t[:], w1f8_d[:])
            xh00 = xhalf_load(0, 0)
            w1t8t = cpool.tile([TILE, 3 * H1], f8)
            nc.sync.dma_start(w1t8t[:], w1t8_d[:])
            sup0 = loads(0, xh00)
            w2f8t = cpool.tile([TILE, 1024], f8)
            nc.sync.dma_start(w2f8t[:], w2f8_d[:])
            w3f8t = cpool.tile([TILE, 256], f8)
            nc.sync.dma_start(w3f8t[:], w3f8_d[:])
            w4t = cpool.tile([TILE, NA2], bf16)
            nc.sync.dma_start(w4t[:], w4p_d[:])
            b4t = cpool.tile([1, NA2], bf16)
            nc.sync.dma_start(b4t[:], b4r_d[:])
            b2t = cpool.tile([TILE, 2], f32)
            nc.sync.dma_start(b2t[:], b2c_d[:])
            b3t = cpool.tile([TILE, 1], f32)
            nc.sync.dma_start(b3t[:], b3r_d[:])
            ones = cpool.tile([1, CHUNK], bf16)
            nc.vector.memset(ones[:], 1.0)

            st = {}

            def s1(bc):
                """L1: 1092 -> 512, fp8 out; b1 rides tl8 row 68."""
                sup = st[bc]["sup"]
                tl8 = sup[1]
                xv = sup[0][bc % hpc][:].rearrange(
                    "q (p i n) -> q p i n", p=4, i=2)
                x1t = apool.tile([TILE, 4 * CHUNK], f8, tag="x1")
                for mp in range(2):
                    ps1 = ps1pool.tile([TILE, 2 * CHUNK], f32, tag="ps1")
                    for mi in range(2):
                        m = mp * 2 + mi
                        om = ps1[:, mi * CHUNK:(mi + 1) * CHUNK]
                        for p in range(4):
                            lhs = w1f8t[:, p * 1024:(p + 1) * 1024].rearrange(
                                "k (i mm) -> k i mm", i=2)[
                                    :, :, m * TILE:(m + 1) * TILE]
                            nc.tensor.matmul(om, lhs, xv[:, p, :, :],
                                             start=(p == 0), stop=False,
                                             perf_mode=DR)
                        # tail also DoubleRow: the zero-weight plane makes
                        # the neighboring chunk's rhs data harmless
                        tb = 0 if (bc % hpc) == 0 else H1
                        lhs_t = w1t8t[:, tb:tb + 2 * H1].rearrange(
                            "k (i mm) -> k i mm", i=2)[
                                :, :, m * TILE:(m + 1) * TILE]
                        nc.tensor.matmul(
                            om, lhs_t, tl8[:].rearrange("k (i n) -> k i n",
                                                        i=2),
                            start=False, stop=True, perf_mode=DR)
                    if mp == 0:
                        nc.scalar.activation(
                            x1t[:, 0:2 * CHUNK], ps1[:], Act.Relu,
                            bias=0.0, scale=1.0 / float(WSCALE))
                    else:
                        # split the second relu ACT/DVE to balance engines
                        nc.scalar.activation(
                            x1t[:, 2 * CHUNK:2 * CHUNK + R1CUT],
                            ps1[:, 0:R1CUT], Act.Relu,
                            bias=0.0, scale=1.0 / float(WSCALE))
                        nc.vector.tensor_scalar(
                            x1t[:, 2 * CHUNK + R1CUT:4 * CHUNK],
                            ps1[:, R1CUT:2 * CHUNK], 1.0 / float(WSCALE),
                            0.0, Alu.mult, Alu.max)
                st[bc]["x1"] = x1t

            def s2(bc):
                """L2: 512 -> 256, fp8 out, bias via ACT bias AP."""
                x1t = st[bc].pop("x1")
                x2t = apool.tile([TILE, 2 * CHUNK], f8, tag="x2")
                for m in range(2):
                    ps2 = ps2pool.tile([TILE, CHUNK], f32, tag="ps2")
                    for c in range(2):
                        lhs = w2f8t[:, c * 512:(c + 1) * 512].rearrange(
                            "k (i mm) -> k i mm", i=2)[
                                :, :, m * TILE:(m + 1) * TILE]
                        rhs = x1t[:, c * 1024:(c + 1) * 1024].rearrange(
                            "k (i n) -> k i n", i=2)
                        nc.tensor.matmul(ps2[:], lhs, rhs, start=(c == 0),
                                         stop=(c == 1), perf_mode=DR)
                    nc.scalar.activation(x2t[:, m * CHUNK:(m + 1) * CHUNK],
                                         ps2[:], Act.Relu,
                                         bias=b2t[:, m:m + 1],
                                         scale=1.0 / float(WSCALE))
                st[bc]["x2"] = x2t

            def s3(bc):
                """L3: 256 -> 128; relu on DVE with per-partition 64*b3 as
                the bias (x3 stays scaled by 64; W4 is pre-divided by 64)."""
                x2t = st[bc].pop("x2")
                ps3 = ps3pool.tile([TILE, CHUNK], f32, tag="ps3")
                nc.tensor.matmul(
                    ps3[:], w3f8t[:].rearrange("k (i mm) -> k i mm", i=2),
                    x2t[:].rearrange("k (i n) -> k i n", i=2),
                    start=True, stop=True, perf_mode=DR)
                x3t = x3pool.tile([TILE, CHUNK], bf16, tag="x3")
                nc.vector.tensor_scalar(x3t[:], ps3[:], b3t[:, 0:1],
                                        0.0, Alu.add, Alu.max)
                st[bc]["x3"] = x3t

            def s5a(bc):
                """L4 logits + fused exp."""
                x3t = st[bc].pop("x3")
                psl = pslpool.tile([TILE, 4 * NA2], f32, tag="psl")
                for s in range(4):
                    om = psl[:, s * NA2:(s + 1) * NA2]
                    nc.tensor.matmul(om, ones[:, 0:TILE], b4t[:],
                                     start=True, stop=False)
                    nc.tensor.matmul(om, x3t[:, s * TILE:(s + 1) * TILE],
                                     w4t[:], start=False, stop=True)
                e16 = ppool.tile([TILE, 4 * NA2], fp16, tag="e16")
                nc.scalar.activation(e16[:], psl[:], Act.Exp,
                                     bias=0.0, scale=1.0)
                st[bc]["e16"] = e16

            def s5b(bc):
                """Softmax normalize + C51 projection + store."""
                sup = st[bc]["sup"]
                idxt, lwet, outsup = sup[2], sup[3], sup[4]
                c4 = bc % hpc
                e16 = st[bc]["e16"]
                ssum = ppool.tile([TILE, 4], f32, tag="ssum")
                nc.vector.tensor_reduce(
                    ssum[:], e16[:].rearrange("q (s a) -> q s a", a=NA2),
                    AX.X, Alu.add)
                inv4 = ppool.tile([TILE, 4], f32, tag="inv4")
                nc.vector.reciprocal(inv4[:], ssum[:])

                # chunk-wide fp16 weight ops; normalize on Pool (has slack)
                p16 = ppool.tile([TILE, 4 * NA2], fp16, tag="p16")
                _p16e = (nc.gpsimd if os.environ.get("K_P16_POOL", "0")
                         == "1" else nc.vector)
                for s in range(4):
                    _p16e.tensor_scalar(
                        p16[:, s * NA2:(s + 1) * NA2],
                        e16[:, s * NA2:(s + 1) * NA2],
                        inv4[:, s:s + 1], None, Alu.mult)
                lw_c = lwet[:, c4 * 4 * NA2:(c4 + 1) * 4 * NA2]
                E_c = lwet[:, kblk + c4 * 4 * NA2:kblk + (c4 + 1) * 4 * NA2]
                wu = ppool.tile([TILE, 4 * NA2], fp16, tag="wu")
                nc.vector.tensor_tensor(wu[:], p16[:], lw_c, Alu.mult)
                wl = ppool.tile([TILE, 4 * NA2], fp16, tag="wl")
                nc.vector.tensor_tensor(wl[:], p16[:], wu[:], Alu.subtract)

                # one fused segmented prefix sum per array (E==0 at each
                # tile's first column resets the carry across tiles);
                # ywu's scan runs on Pool to offload DVE
                ywl = ppool.tile([TILE, 4 * NA2], fp16, tag="ywl")
                nc.vector.tensor_tensor_scan(ywl[:], E_c, wl[:], 0.0,
                                             Alu.mult, Alu.add)
                ywu = ppool.tile([TILE, 4 * NA2], fp16, tag="ywu")
                _scanu = (nc.gpsimd if os.environ.get("K_SCANU_POOL", "0")
                          == "1" else nc.vector)
                _scanu.tensor_tensor_scan(ywu[:], E_c, wu[:], 0.0,
                                          Alu.mult, Alu.add)
                # host pre-offset indices by s*102: one scatter per array
                scl = ppool.tile([TILE, 4 * NA2], fp16, tag="scl")
                scu = ppool.tile([TILE, 4 * NA2], fp16, tag="scu")
                cb = c4 * 4 * NA2
                nc.gpsimd.local_scatter(
                    scl[:], ywl[:], idxt[:, cb:cb + 4 * NA2],
                    channels=TILE, num_elems=4 * NA2, num_idxs=4 * NA2)
                nc.gpsimd.local_scatter(
                    scu[:], ywu[:], idxt[:, kblk + cb:kblk + cb + 4 * NA2],
                    channels=TILE, num_elems=4 * NA2, num_idxs=4 * NA2)
                _comb = (nc.gpsimd if os.environ.get("K_COMB_POOL", "1")
                         == "1" else nc.vector)
                _comb.tensor_tensor(
                    outsup[:, c4 * 4 * NA:(c4 + 1) * 4 * NA].rearrange(
                        "q (s a) -> q s a", a=NA),
                    scl[:].rearrange("q (s a) -> q s a", a=NA2)[:, :, 0:NA],
                    scu[:].rearrange("q (s a) -> q s a", a=NA2)[:, :, 0:NA],
                    Alu.add)
                if c4 == hpc - 1:
                    sb = bc // hpc
                    nc.sync.dma_start(
                        out_d[:, sb * tps * NA:(sb + 1) * tps * NA],
                        outsup[:])
                del st[bc]

            # ---- depth-4 layer-pipelined loop; loads issue first so
            # stores never block them on the in-order SP queue ----
            sup = sup0
            for it in range(n_chunks + 3):
                if it < n_chunks:
                    if it % hpc == 0 and it > 0:
                        sup = loads(it // hpc)
                    st[it] = {"sup": sup}
                if it - 3 >= 0:
                    s5a(it - 3)
                if it - 2 >= 0 and it - 2 < n_chunks:
                    s3(it - 2)
                if it - 3 >= 0:
                    s5b(it - 3)
                if it - 1 >= 0 and it - 1 < n_chunks:
                    s2(it - 1)
                if it < n_chunks:
                    s1(it)

    nc.compile()
    return nc


# ------------------------- host side -------------------------

def _host_prep(obs, actions, rewards, bootstrap, discount, q_support,
               W1, b1, W2, b2, W3, b3, W4, b4, n_rows=B_CORE):
    B = obs.shape[0]
    nt = n_rows // TILE
    tps = SUPER // TILE
    nsup = n_rows // SUPER
    g = (bootstrap * discount).astype(F32)

    # ---- projection structure (continuous in b; no bit-exactness needed) --
    tz = rewards[:, None] + g[:, None] * q_support[None, :].astype(F32)
    tz = np.clip(tz.astype(F32), F32(-10.0), F32(10.0))
    bh = ((tz + F32(10.0)) * F32(5.0)).astype(F32)          # [B,101] in [0,100]
    li = np.floor(bh)
    lw = np.zeros((B, NA2), FP16)
    lw[:, :NA] = (bh - li).astype(FP16)
    E = np.zeros((B, NA2), FP16)
    E[:, 1:NA] = (li[:, 1:] == li[:, :-1]).astype(FP16)     # run continues
    lm = np.ones((B, NA), bool)
    lm[:, :-1] = li[:, :-1] != li[:, 1:]                    # run ends
    lii = li.astype(np.int16)
    idxl = np.full((B, NA2), -1, np.int16)
    idxu = np.full((B, NA2), -1, np.int16)
    idxl[:, :NA] = np.where(lm, lii, np.int16(-1))
    idxu[:, :NA] = np.where(lm, lii + np.int16(1), np.int16(-1))
    g0 = g == 0
    idxl[g0] = -1                                           # host handles g==0
    idxu[g0] = -1
    # chunk-batched scatter: tile s of a chunk scatters into [s*102,s*102+101]
    soff = (((np.arange(B) // TILE) % 4).astype(np.int16)
            * np.int16(NA2))[:, None]
    idxl = np.where(idxl >= 0, idxl + soff, idxl)
    idxu = np.where(idxu >= 0, idxu + soff, idxu)

    def blockpack(a, b_, s):
        # [n_rows, NA2] x2 -> [128, nsup * 2 * tps * NA2], per-super blocks
        aa = a[s].reshape(nsup, tps, TILE, NA2).transpose(2, 0, 1, 3)
        bb = b_[s].reshape(nsup, tps, TILE, NA2).transpose(2, 0, 1, 3)
        st = np.stack([aa, bb], axis=2)          # [q, sb, 2, ts, NA2]
        return np.ascontiguousarray(st.reshape(TILE, -1))

    # ---- MLP weights (fp8, x64 for subnormal headroom) ----
    w1f8 = np.ascontiguousarray(
        (W1[:1024] * WSCALE).astype(FP8)
        .reshape(4, 2, TILE, H1).transpose(2, 0, 1, 3).reshape(TILE, 4096))
    W1tail = np.zeros((TILE, H1), F32)
    W1tail[:68] = W1[1024:1092]
    W1tail[68] = b1
    wt = (W1tail * WSCALE).astype(FP8)
    w1t8 = np.concatenate([wt, np.zeros((TILE, H1), FP8), wt], axis=1)
    w2f8 = np.ascontiguousarray(
        (W2 * WSCALE).astype(FP8)
        .reshape(2, 2, TILE, H2).transpose(2, 0, 1, 3).reshape(TILE, 1024))
    w3f8 = np.ascontiguousarray(
        (W3 * WSCALE).astype(FP8)
        .reshape(2, TILE, H3).transpose(1, 0, 2).reshape(TILE, 256))
    w4p = np.zeros((TILE, NA2), BF16)
    w4p[:, :NA] = (W4 / WSCALE).astype(BF16)     # x3 is scaled by 64
    b4r = np.full((1, NA2), F32(-30.0), F32)                # pad logit -> -30
    b4r[0, :NA] = b4
    b4r = b4r.astype(BF16)
    b2c = np.ascontiguousarray(b2.reshape(2, TILE).T).astype(F32)
    b3r = (b3 * WSCALE)[:, None].astype(F32)

    # ---- activations, feature-major fp8 ----
    xt8_all = np.ascontiguousarray(obs[:, :1024].astype(FP8).T)   # [1024, B]
    tail_all = np.zeros((TILE, B), FP8)
    tail_all[:66] = obs[:, 1024:1090].T.astype(FP8)
    tail_all[66:68] = actions.T.astype(FP8)
    tail_all[68] = FP8(1.0)                                 # b1 rider row

    shared = dict(w1f8=w1f8, w1t8=w1t8, w2f8=w2f8, w3f8=w3f8,
                  w4p=w4p, b4r=b4r, b2c=b2c, b3c64=b3r)
    in_maps = []
    for c in range(B // n_rows):
        s = slice(c * n_rows, (c + 1) * n_rows)
        m = dict(shared)
        m["xt8"] = np.ascontiguousarray(xt8_all[:, s])
        m["tail8"] = np.ascontiguousarray(tail_all[:, s])
        m["idxpk"] = blockpack(idxl, idxu, s)
        m["lwepk"] = blockpack(lw, E, s)
        in_maps.append(m)
    return in_maps, g


def _host_g0(out, rewards, g):
    """Closed-form output for bootstrap==0 rows (independent of the MLP:
    b is constant across atoms, probabilities sum to 1)."""
    rows = np.nonzero(g == 0)[0]
    if rows.size == 0:
        return out
    r = np.clip(rewards[rows], F32(-10.0), F32(10.0)).astype(F32)
    b0 = ((r + F32(10.0)) * F32(5.0)).astype(F32)
    li = np.floor(b0)
    frac = (b0 - li).astype(F32)
    ii = li.astype(np.int64)
    ni = frac > 0
    np.add.at(out, (rows[ni], ii[ni]), (F32(1.0) - frac[ni]))
    np.add.at(out, (rows[ni], ii[ni] + 1), frac[ni])
    isint = ~ni
    interior = isint & (ii > 0) & (ii < 100)
    np.add.at(out, (rows[interior], ii[interior] - 1), F32(1.0))
    np.add.at(out, (rows[interior], ii[interior] + 1), F32(1.0))
    edge = isint & ~interior
    np.add.at(out, (rows[edge], ii[edge]), F32(1.0))
    return out


_NC_CACHE = {}


def kernel(obs, actions, rewards, bootstrap, discount, q_support,
           W1, b1, W2, b2, W3, b3, W4, b4):
    obs = np.asarray(obs, F32)
    actions = np.asarray(actions, F32)
    rewards = np.asarray(rewards, F32)
    bootstrap = np.asarray(bootstrap, F32)
    discount = np.asarray(discount, F32)
    q_support = np.asarray(q_support, F32)
    W1, b1 = np.asarray(W1, F32), np.asarray(b1, F32)
    W2, b2 = np.asarray(W2, F32), np.asarray(b2, F32)
    W3, b3 = np.asarray(W3, F32), np.asarray(b3, F32)
    W4, b4 = np.asarray(W4, F32), np.asarray(b4, F32)
    assert obs.shape == (B_FULL, D_OBS) and actions.shape == (B_FULL, 2)

    in_maps, g = _host_prep(
        obs, actions, rewards, bootstrap, discount, q_support,
        W1, b1, W2, b2, W3, b3, W4, b4)

    if B_CORE not in _NC_CACHE:
        _NC_CACHE[B_CORE] = build_nc(B_CORE)
    nc = _NC_CACHE[B_CORE]

    trace = bool(int(os.environ.get("KERNEL_TRACE", "0")))
    res = run_bass_kernel_spmd(nc, in_maps, list(range(N_CORES)), trace=trace)
    kernel.last_results = res

    nt = B_CORE // TILE
    out = np.concatenate(
        [r["out"].reshape(TILE, nt, NA).transpose(1, 0, 2)
         .reshape(B_CORE, NA).astype(F32) for r in res.results], axis=0)
    out = _host_g0(out, rewards, g)
    return out

